# revision 62
# baseline (speedup 1.0000x reference)
"""Trainium2 Bass kernel for a 2-layer LSTM binary classifier.

Model: xp0 = x @ Wih0.T + b0 ; layer0 LSTM ; xp1 = seq0 @ Wih1.T + b1 ;
layer1 LSTM ; out = h1_T @ Wfc.T + bfc.

Sharding: data-parallel over batch (64 -> 8 cores x 8 examples), all
weights replicated.  Per core:
  Phase 1 (interleaved with phase 2): big input GEMM in float32r (full PE
    rate at N>=256, ~tf32 precision), bias added via K=1 ones-matmuls,
    output xp0 stored in SBUF as [128(gate-unit), gate, t, b].
  Phase 2: serial recurrence, the wall-clock driver (~1.9us/step chain
    latency x 258 macro-steps).  Gates live as [gate-dim on partitions,
    batch on free].  Per macro-step u: layer0 runs step u and layer1 runs
    step u-LAG so both layers share joint elementwise instructions.
    xp contributions are pre-accumulated into PSUM per W-step window
    (identity matmul for layer0's xp0; a single K=4 gate-onehot matmul
    broadcasts layer1's bias and must be the only start=True write to the
    bank -- start=True clears the whole bank's has_written bits).  Wih1 @
    h0(v) runs as per-step matmuls that depend on LAG-old data, so they
    execute off the critical path.  tanh-gate weights are pre-scaled 2x on
    the host so sigmoid covers the g-gate too (tanh(a) = 2*sig(2a)-1,
    fixed up inside fused scalar_tensor_tensor DVE ops); the o-gate's
    sigmoid is a separate ACT op because it is only needed at the chain
    tail, so the on-chain sigmoid(i,f,g) is gated by just 6 of 8 matmuls.
    Per-step chain: 6 bf16 matmuls -> sigmoid_ifg (ACT) -> 3 DVE ops ->
    tanh (ACT) -> h-mul (DVE, bf16 out) -> next step's matmuls; ~1.79us
    of which ~1us is cross-engine sem/pipeline-drain latency.
"""

import numpy as np
import ml_dtypes

import concourse.bass as bass
import concourse.tile as tile
from concourse import bacc, mybir
from concourse.bass_utils import run_bass_kernel_spmd

F32 = mybir.dt.float32
F32R = mybir.dt.float32r
BF16 = mybir.dt.bfloat16
AF = mybir.ActivationFunctionType

H = 128          # hidden
D = 2048         # input size
B = 64           # batch
T = 256          # seq len
NCORES = 8
BS = B // NCORES          # 8 examples per core
KT = D // 128             # 16 k-tiles of the input GEMM
NCHUNK = 4                # GEMM token chunks
CTOK = T * BS // NCHUNK   # 512 tokens per chunk
TW = CTOK // BS           # 64 timesteps per chunk
W = 8                     # recurrence window (psum burst granularity)
NW = T // W
LAG = 2                   # layer1 runs LAG steps behind layer0
GORD = [0, 1, 2, 3]       # our gate order [i,f,g,o] -> torch block index


def _build_phase2_step(nc, u, P, hwin, hinit, whh0t_s, whh1t_s, wih1t_s,
                       sig, fcT, igT, tcT, cC):
    """Emit one macro-step: layer0 step u, layer1 step u-LAG."""
    w, s = divmod(u, W)
    active = []
    if u < T:
        active.append(0)
    if u >= LAG:
        active.append(1)

    def hprev(layer, step):
        if step == 0:
            return hinit[:, layer, :]
        pu = step - 1 + (LAG if layer == 1 else 0)
        return hwin[:, (pu // W) % 2, pu % W, layer, :]

    # layer1 input projection for step v=u-LAG: depends on h0(v), which was
    # produced LAG steps ago -> executes early on PE, off the critical path
    if 1 in active:
        v = u - LAG
        h0v = hwin[:, (v // W) % 2, v % W, 0, :]
        for gi in range(4):
            nc.tensor.matmul(P[:, 1, gi, s, :], wih1t_s[:, gi, :],
                             h0v, start=False, stop=False,
                             skip_group_check=True)

    # step matmuls
    for gi in (0, 1, 2, 3):
        for l in active:
            st = u if l == 0 else u - LAG
            lhs = whh0t_s if l == 0 else whh1t_s
            nc.tensor.matmul(P[:, l, gi, s, :], lhs[:, gi, :], hprev(l, st),
                             start=False, stop=True, skip_group_check=True)

    lo = active[0]
    ln = len(active)
    L = slice(lo, lo + ln)
    # g-gate weights/bias pre-scaled by 2 on host: tanh(a) = 2*sigmoid(2a)-1,
    # so ONE sigmoid covers all 4 gates; the 2x-1 fixup fuses into the
    # scalar_tensor_tensor c-update:
    #   c = f*c + i*(2*sg-1) = fc + 2*(i*(sg-0.5))
    nc.scalar.activation(sig[:, L, 0:3, :], P[:, L, 0:3, s, :], AF.Sigmoid)
    nc.scalar.activation(sig[:, L, 3, :], P[:, L, 3, s, :], AF.Sigmoid)
    nc.vector.tensor_mul(fcT[:, L, :], sig[:, L, 1, :], cC[:, L, :])
    nc.vector.scalar_tensor_tensor(
        igT[:, L, :], sig[:, L, 2, :], 0.5, sig[:, L, 0, :],
        op0=mybir.AluOpType.subtract, op1=mybir.AluOpType.mult)
    nc.vector.scalar_tensor_tensor(
        cC[:, L, :], igT[:, L, :], 2.0, fcT[:, L, :],
        op0=mybir.AluOpType.mult, op1=mybir.AluOpType.add)
    nc.scalar.activation(tcT[:, L, :], cC[:, L, :], AF.Tanh)
    nc.vector.tensor_mul(hwin[:, w % 2, s, L, :], sig[:, L, 3, :], tcT[:, L, :])


def build_program(debug_taps=False):
    nc = bacc.Bacc("TRN2", target_bir_lowering=False, debug=False,
                   enable_asserts=False)

    # ---- DRAM I/O ----
    xd = nc.dram_tensor("xp", [NCHUNK, 128, KT, CTOK], F32R,
                        kind="ExternalInput").ap()
    wih0d = nc.dram_tensor("wih0t", [128, KT, 4, 128], F32R,
                           kind="ExternalInput").ap()
    wih0bfd = nc.dram_tensor("wih0bf", [128, KT, 4, 128], BF16,
                             kind="ExternalInput").ap()
    x0bfd = nc.dram_tensor("x0bf", [128, KT, CTOK], BF16,
                           kind="ExternalInput").ap()
    whh0d = nc.dram_tensor("whh0t", [128, 4, 128], BF16,
                           kind="ExternalInput").ap()
    whh1d = nc.dram_tensor("whh1t", [128, 4, 128], BF16,
                           kind="ExternalInput").ap()
    wih1d = nc.dram_tensor("wih1t", [128, 4, 128], BF16,
                           kind="ExternalInput").ap()
    b0d = nc.dram_tensor("b0s", [1, 4, 128], BF16, kind="ExternalInput").ap()
    b1d = nc.dram_tensor("b1g4", [4, 128], BF16, kind="ExternalInput").ap()
    onesd = nc.dram_tensor("ones512", [1, CTOK], BF16,
                           kind="ExternalInput").ap()
    onehotd = nc.dram_tensor("onehot4", [4, 4, W * BS], BF16,
                             kind="ExternalInput").ap()
    identd = nc.dram_tensor("ident", [128, 128], F32,
                            kind="ExternalInput").ap()
    wfcd = nc.dram_tensor("wfct", [128, 1], BF16, kind="ExternalInput").ap()
    bfcd = nc.dram_tensor("bfcb", [BS, 1], F32, kind="ExternalInput").ap()
    yd = nc.dram_tensor("y", [BS, 1], F32, kind="ExternalOutput").ap()
    if debug_taps:
        dbg_xp0 = nc.dram_tensor("dbg_xp0", [128, 4, TW, BS], F32,
                                 kind="ExternalOutput").ap()
        dbg_hwin = nc.dram_tensor("dbg_hwin", [128, 2, W, 2, BS], BF16,
                                  kind="ExternalOutput").ap()
        dbg_c = nc.dram_tensor("dbg_c", [128, 2, BS], F32,
                               kind="ExternalOutput").ap()

    with tile.TileContext(nc) as tc, \
            tc.tile_pool(name="persist", bufs=1) as pp:
        # ---- persistent SBUF ----
        wih0t_s = pp.tile([128, KT, 4, 128], F32R, name="wih0t_s")
        wih0bf_s = pp.tile([128, KT, 4, 128], BF16, name="wih0bf_s")
        x0bf_s = pp.tile([128, KT, CTOK], BF16, name="x0bf_s")
        whh0t_s = pp.tile([128, 4, 128], BF16, name="whh0t_s")
        whh1t_s = pp.tile([128, 4, 128], BF16, name="whh1t_s")
        wih1t_s = pp.tile([128, 4, 128], BF16, name="wih1t_s")
        b0s_s = pp.tile([1, 4, 128], BF16, name="b0s_s")
        b1s_s = pp.tile([4, 128], BF16, name="b1s_s")
        ones_s = pp.tile([1, CTOK], BF16, name="ones_s")
        onehot_s = pp.tile([4, 4, W * BS], BF16, name="onehot_s")
        ident_s = pp.tile([128, 128], F32, name="ident_s")
        wfct_s = pp.tile([128, 1], BF16, name="wfct_s")
        bfcb_s = pp.tile([BS, 1], F32, name="bfcb_s")

        # only the GEMM-bias constants must precede chunk-0's x DMAs on the
        # sync queue; everything else is issued after the x prologue so the
        # first-window critical path isn't stuck behind their queue slots
        nc.sync.dma_start(b0s_s[:], b0d[:])
        nc.sync.dma_start(ones_s[:], onesd[:])
        for _k2 in range(0, KT, 2):
            nc.gpsimd.dma_start(wih0bf_s[:, _k2:_k2 + 2],
                                wih0bfd[:, _k2:_k2 + 2])
        for _k in range(KT):
            nc.gpsimd.dma_start(wih0t_s[:, _k], wih0d[:, _k])

        def _late_const_dmas():
            nc.sync.dma_start(whh0t_s[:], whh0d[:])
            nc.sync.dma_start(whh1t_s[:], whh1d[:])
            nc.sync.dma_start(wih1t_s[:], wih1d[:])
            nc.sync.dma_start(b1s_s[:], b1d[:])
            nc.sync.dma_start(onehot_s[:], onehotd[:])
            nc.sync.dma_start(ident_s[:], identd[:])
            nc.sync.dma_start(wfct_s[:], wfcd[:])
            nc.sync.dma_start(bfcb_s[:], bfcd[:])

        # xp0 per chunk: [128, gate, t-local, b] fp32
        xp0_t = [pp.tile([128, 4, TW, BS], F32, name=f"xp0_{c}")
                 for c in range(NCHUNK)]

        # recurrence state
        cC = pp.tile([128, 2, BS], F32, name="cC")
        hinit = pp.tile([128, 2, BS], BF16, name="hinit")
        hwin = pp.tile([128, 2, W, 2, BS], BF16, name="hwin")
        sig = pp.tile([128, 2, 4, BS], F32, name="sig")
        fcT = pp.tile([128, 2, BS], F32, name="fcT")
        igT = pp.tile([128, 2, BS], F32, name="igT")
        tcT = pp.tile([128, 2, BS], F32, name="tcT")
        y_sb = pp.tile([BS, 1], F32, name="y_sb")

        nc.vector.memset(cC[:], 0.0)
        nc.vector.memset(hinit[:], 0.0)
        # pre-warm the ACT function table (LoadActFuncSet ~1.3us) off-chain
        nc.scalar.activation(tcT[:, 0:1, :], cC[:, 0:1, :], AF.Sigmoid)
        nc.scalar.activation(tcT[:, 0:1, :], cC[:, 0:1, :], AF.Tanh)

        with (
            tc.tile_pool(name="xchunk", bufs=2) as x_pool,
            tc.tile_pool(name="gemm_ps", bufs=4, space="PSUM") as gemm_ps,
            tc.tile_pool(name="pair_ps", bufs=2, space="PSUM") as pair_ps,
        ):
            # ---- GEMM op generator (pulled incrementally) ----
            def gemm_gen():
                for c in range(NCHUNK):
                    # chunk 0 runs fully in bf16 (weights + x) so the first
                    # window needs only ~3MB of startup DMA; its xp0 error
                    # (first 64 steps) is decayed away by the forget gates.
                    # Later chunks use f32r.  Two half-token passes so the
                    # first recurrence window unblocks early; k-outer so
                    # all 4 gates finish together.
                    if c == 0:
                        xt, wmat = x0bf_s, wih0bf_s
                    else:
                        xt = x_pool.tile([128, KT, CTOK], F32R, name="xt")
                        wmat = wih0t_s
                    halves = 2 if c == 0 else 1
                    hw_ = CTOK // halves
                    tw_ = hw_ // BS
                    for hp in range(halves):
                        tsl = slice(hp * hw_, (hp + 1) * hw_)
                        for k in range(KT):
                            eng = nc.sync
                            if c == 0:
                                eng.dma_start(xt[:, k, tsl],
                                              x0bfd[:, k, tsl])
                            else:
                                eng.dma_start(xt[:, k, tsl], xd[c, :, k, tsl])
                            yield 1
                        pg = []
                        for g in range(4):
                            p = gemm_ps.tile([128, CTOK], F32, name="pg")
                            pg.append(p)
                            nc.tensor.matmul(p[:, 0:hw_], b0s_s[:, g, :],
                                             ones_s[:, 0:hw_],
                                             start=True, stop=False,
                                             skip_group_check=True)
                            yield 1
                        for k in range(KT):
                            for g in range(4):
                                nc.tensor.matmul(
                                    pg[g][:, 0:hw_], wmat[:, k, g, :],
                                    xt[:, k, tsl],
                                    start=False, stop=(k == KT - 1),
                                    skip_group_check=True)
                                yield 1
                        for g in range(4):
                            dst = xp0_t[c][:, g, hp * tw_:(hp + 1) * tw_, :]
                            srcv = pg[g][:, 0:hw_].rearrange(
                                "p (t b) -> p t b", t=tw_)
                            nc.scalar.copy(dst, srcv)
                            yield 1

            gen = gemm_gen()

            def pull(n):
                for _ in range(n):
                    if next(gen, None) is None:
                        break

            # prologue: chunk-0 first-half pass + second-half dmas only --
            # the recurrence's first window must enter the PE queue BEFORE
            # the second half-pass matmuls (which pace with their DMAs)
            pull(KT)
            _late_const_dmas()
            pull(4 + 4 * KT + 4)

            P = None
            for u in range(T + LAG):
                w, s = divmod(u, W)
                if s == 0:
                    P = pair_ps.tile([128, 2, 4, 16, BS], F32, name="pairP")
                    if u < T:
                        c, lw = divmod(w, TW // W)
                        nc.tensor.matmul(
                            P[:, 0, :, 0:W, :],
                            ident_s[:, :],
                            xp0_t[c][:, :, lw * W:(lw + 1) * W, :],
                            start=True, stop=False, skip_group_check=True)
                    if u + W > LAG:
                        # whole-bank bias broadcast in ONE start=True matmul
                        # (start=True clears has_written for the full bank);
                        # Wih1 @ h0 is added per-step (off the critical path).
                        nc.tensor.matmul(
                            P[:, 1, :, 0:W, :], b1s_s[:, :], onehot_s[:, :, :],
                            start=True, stop=False, skip_group_check=True)
                # delay GEMM-op interleave a few steps so the in-order PE
                # queue never stalls on a matmul whose x-slab DMA is still
                # in flight
                if u >= 2:
                    pull(4)
                _build_phase2_step(nc, u, P, hwin, hinit, whh0t_s, whh1t_s,
                                   wih1t_s, sig, fcT, igT, tcT, cC)
                if debug_taps and u == 31:
                    nc.sync.dma_start(dbg_xp0[:], xp0_t[0][:])
                    nc.sync.dma_start(dbg_hwin[:], hwin[:])
                    nc.sync.dma_start(dbg_c[:], cC[:])

            pull(10000)  # drain any leftovers (shouldn't be needed)

            # ---- final fc ----
            fcp = gemm_ps.tile([BS, 1], F32, name="pg")
            nc.tensor.matmul(fcp[:, :], hwin[:, (T + LAG - 1) // W % 2,
                                             (T + LAG - 1) % W, 1, :],
                             wfct_s[:, :], start=True, stop=True,
                             skip_group_check=True)
            nc.scalar.activation(y_sb[:, :], fcp[:, :], AF.Identity,
                                 bias=bfcb_s[:, :])
            nc.sync.dma_start(yd[:], y_sb[:])

    nc.compile()
    return nc


_PROG = None


def _get_program():
    global _PROG
    if _PROG is None:
        _PROG = build_program()
    return _PROG


def prep_inputs(x, Wih0, Whh0, bih0, bhh0, Wih1, Whh1, bih1, bhh1, Wfc, bfc):
    """Host-side layout prep -> per-core in_maps."""
    bf = ml_dtypes.bfloat16
    x = np.asarray(x, np.float32)

    # weights: [4H, K] -> [K(part), gate(ours), unit]
    def gate_T(Wmat):  # [512, K] -> [K, 4, 128] in our gate order
        A = np.asarray(Wmat, np.float32).reshape(4, 128, -1)  # tg, j, k
        A = A.transpose(2, 0, 1)[:, GORD, :]                  # k, ours, j
        A = A.copy()
        A[:, 2, :] *= 2.0  # tanh-gate folded 2x (tanh(a)=2*sig(2a)-1)
        return np.ascontiguousarray(A)

    wih0t = gate_T(Wih0).reshape(KT, 128, 4, 128).transpose(1, 0, 2, 3)
    wih0t = np.ascontiguousarray(wih0t, np.float32)           # [128,KT,4,128]
    whh0t = gate_T(Whh0).astype(bf)                           # [128,4,128]
    whh1t = gate_T(Whh1).astype(bf)
    wih1t = gate_T(Wih1).astype(bf)

    b0 = (np.asarray(bih0) + np.asarray(bhh0)).astype(np.float32)
    b1 = (np.asarray(bih1) + np.asarray(bhh1)).astype(np.float32)
    b0s = b0.reshape(4, 128)[GORD].copy()
    b0s[2] *= 2.0
    b0s = b0s[None].astype(bf)                                # [1,4,128]
    b1g4 = b1.reshape(4, 128)[GORD].copy()
    b1g4[2] *= 2.0
    b1g4 = b1g4.astype(bf)                                    # [4,128]
    onehot4 = np.einsum("kg,n->kgn", np.eye(4, dtype=np.float32),
                        np.ones(W * BS, np.float32)).astype(bf)
    ones512 = np.ones((1, CTOK), bf)
    ident = np.eye(128, dtype=np.float32)
    wfct = np.asarray(Wfc, np.float32).T.astype(bf)           # [128,1]
    bfcb = np.full((BS, 1), np.asarray(bfc, np.float32)[0], np.float32)

    wih0bf = wih0t.astype(bf)
    common = dict(wih0t=wih0t, wih0bf=wih0bf,
                  whh0t=whh0t, whh1t=whh1t, wih1t=wih1t,
                  b0s=b0s, b1g4=b1g4, onehot4=onehot4, ones512=ones512,
                  ident=ident, wfct=wfct, bfcb=bfcb)

    in_maps = []
    for c in range(NCORES):
        xs = x[c * BS:(c + 1) * BS]                           # [BS, T, D]
        xt = xs.transpose(2, 1, 0).reshape(D, T * BS)         # [d, tok(t,b)]
        xpre = (xt.reshape(KT, 128, NCHUNK, CTOK)
                .transpose(2, 1, 0, 3))                       # [c,128,k,tok]
        in_maps.append({"xp": np.ascontiguousarray(xpre, np.float32),
                        "x0bf": xpre[0].astype(bf), **common})
    return in_maps


def run(inputs, **kw):
    nc = _get_program()
    in_maps = prep_inputs(**inputs)
    res = run_bass_kernel_spmd(nc, in_maps, core_ids=list(range(NCORES)), **kw)
    y = np.concatenate([res.results[c]["y"] for c in range(NCORES)], axis=0)
    return y.astype(np.float32), res


def kernel(**inputs):
    y, _ = run(inputs)
    return y


if __name__ == "__main__":
    import sys
    if "--sim" in sys.argv:
        import trails.perfetto as _tp
        if not hasattr(_tp.LazyPerfetto, "add_counter"):
            def _add_counter(self, proc, track, ts_, val):
                self.update_counter(proc, track, int(ts_), float(val),
                                    unit="ns")
            _tp.LazyPerfetto.add_counter = _add_counter
        for _m in ("enable_explicit_ordering", "reserve_process_order"):
            if not hasattr(_tp.LazyPerfetto, _m):
                setattr(_tp.LazyPerfetto, _m,
                        lambda self, *a, **k: None)
        from concourse.timeline_sim import TimelineSim
        nc = _get_program()
        ts = TimelineSim(nc, trace="--trace" in sys.argv)
        dur = ts.simulate()
        print(f"TimelineSim predicted duration: {dur:.0f} ns")
        if ts.perfetto is not None:
            ts.perfetto.save("/root/problem/timeline.pftrace")
            print("wrote /root/problem/timeline.pftrace")
